# revision 49
# baseline (speedup 1.0000x reference)
"""Trainium2 Bass kernel for nn_ACPClassifier (RetNet-style block + classifier).

v7 design (~131us vs 200-238us v2 baseline). Key points:
- Only the last vocab iteration matters (x overwritten each pass); data
  parallel over batch, one batch element per NeuronCore.
- Host: embedding gather + pos add + LN1 (same DMA bytes as shipping emb);
  final sigmoid on host (kernel returns the logit).
- All seven projections run as fp8e4 DoubleRow matmuls (weights x256,
  descale folded into the PSUM-evac activation scale): 2 MMs per 512-deep
  contraction instead of 4. Attention stays fp16.
- Act-table discipline: Silu gates + reciprocal_approx_fast(DVE)+Sqrt for
  every rstd -> 3 table loads total (v2 had 29 = 45us); silu set primed at
  t=0 via a 1-element dummy op on the vector+scalar queues.
- Residual adds folded into projection matmul groups via scaled-identity
  lhsT (identR=256*I, identF=4096*I match the fp8 scale chain).
- Retention state recurrence on the PE: per-pair merged KV matmul plus a
  diagonal-decay matmul (gamma^128 per head), one state-evac copy per block.
  Score/cross MMs ordered so 64-row halves land in disjoint PE row/col
  groups and run concurrently.
- Group norm with uncentered stats: y and y^2 both evacuated from PSUM
  during the attention phase, so both hpair stat matmuls fire at tail start.
- LN2/LN3 sumsq via fp8 jones DoubleRow on Square(xc/2) with 4/D var
  rescale; LN3 folded into the classifier dot on centered x (no wfc term).
- DMAs spread across sync/scalar/gpsimd queues; chunk-1 projections
  interleaved with chunk-0 attention to keep the PE HAM-warm.
"""

import numpy as np

import concourse.bacc as bacc
import concourse.mybir as mybir
from concourse.bass_utils import run_bass_kernel_spmd
from concourse.tile import TileContext

F32 = mybir.dt.float32
F16 = mybir.dt.float16
F8 = mybir.dt.float8e4
DR = mybir.MatmulPerfMode.DoubleRow
WS = 256.0   # fp8 weight scale
HS = 16.0    # h1 activation scale
AF = mybir.ActivationFunctionType
OP = mybir.AluOpType

DIM, SEQ, HEADS, HDIM, BATCH, VOCAB, NVOCAB = 512, 1024, 8, 64, 8, 1024, 3
EPS = 1e-5
P = 128
NDB = DIM // P    # 4 d-blocks (also head pairs)
NTB = SEQ // P    # 8 token blocks
NCH = SEQ // 512  # 2 free-dim chunks of 512
NPAIR = 4
NCORES = 8


def build_nc():
    nc = bacc.Bacc(
        "TRN2",
        target_bir_lowering=False,
        debug=False,
        enable_asserts=False,
        num_devices=NCORES,
    )

    # ---- DRAM parameters (per-core inputs) ----
    d_xh1 = nc.declare_dram_parameter("xh1", [P, NDB, SEQ], F16, isOutput=False)
    d_w = {
        k: nc.declare_dram_parameter(k, [P, NDB, DIM], F8, isOutput=False)
        for k in ("wq", "wk", "wv", "wg", "wo", "w1", "w2")
    }
    d_xh18 = nc.declare_dram_parameter("xh18", [P, NDB, SEQ], F8, isOutput=False)
    d_qdec = nc.declare_dram_parameter("qdec", [P, NDB, SEQ], F16, isOutput=False)
    d_gdk = nc.declare_dram_parameter("gdk", [P, DIM], F16, isOutput=False)
    d_mdec = nc.declare_dram_parameter("mdec", [P, 2, 4 * P], F16, isOutput=False)
    d_hpair = nc.declare_dram_parameter("hpair", [P, P], F16, isOutput=False)
    d_dg = nc.declare_dram_parameter("dg", [P, NPAIR, P], F16, isOutput=False)
    d_identR = nc.declare_dram_parameter("identR", [P, P], F16, isOutput=False)
    d_identF = nc.declare_dram_parameter("identF", [P, P], F16, isOutput=False)
    d_fct = nc.declare_dram_parameter("fcT", [P, NDB, SEQ], F16, isOutput=False)
    d_b1 = nc.declare_dram_parameter("b1t", [P, NDB], F32, isOutput=False)
    d_b2 = nc.declare_dram_parameter("b2t", [P, NDB], F32, isOutput=False)
    d_fcb = nc.declare_dram_parameter("fcb", [1, 1], F32, isOutput=False)
    d_out = nc.declare_dram_parameter("out", [1, 1], F32, isOutput=True)

    with TileContext(nc) as tc:
        from contextlib import ExitStack

        ctx = ExitStack()
        with ctx:
            acts = ctx.enter_context(tc.tile_pool(name="acts", bufs=1))
            wts = ctx.enter_context(tc.tile_pool(name="wts", bufs=1))
            smal = ctx.enter_context(tc.tile_pool(name="smal", bufs=1))
            pp = ctx.enter_context(tc.tile_pool(name="pp", bufs=1, space="PSUM"))

            # ---- weights on the projq critical path first ----
            def load_w(key):
                t = wts.tile([P, NDB, DIM], F8, tag=f"t_w_{key}",
                             name=f"w_{key}")
                nc.sync.dma_start(t[:], d_w[key][:])
                return t

            wq_t = load_w("wq")
            xh18 = acts.tile([P, NDB, SEQ], F8, tag="t_xh18", name="xh18")
            nc.scalar.dma_start(xh18[:], d_xh18[:])
            xh1 = acts.tile([P, NDB, SEQ], F16, tag="t_xh1", name="xh1")
            nc.gpsimd.dma_start(xh1[:], d_xh1[:])
            wk_t = wts.tile([P, NDB, DIM], F8, tag="t_w_wk", name="w_wk")
            nc.gpsimd.dma_start(wk_t[:], d_w["wk"][:])
            wv_t = wts.tile([P, NDB, DIM], F8, tag="t_w_wv", name="w_wv")
            nc.scalar.dma_start(wv_t[:], d_w["wv"][:])
            wg_t = wts.tile([P, NDB, DIM], F8, tag="t_w_wg", name="w_wg")
            nc.sync.dma_start(wg_t[:], d_w["wg"][:])
            wo_t = wts.tile([P, NDB, DIM], F8, tag="t_w_wo", name="w_wo")
            nc.scalar.dma_start(wo_t[:], d_w["wo"][:])
            w1_t = wts.tile([P, NDB, DIM], F8, tag="t_w_w1", name="w_w1")
            nc.sync.dma_start(w1_t[:], d_w["w1"][:])
            w2_t = wts.tile([P, NDB, DIM], F8, tag="t_w_w2", name="w_w2")
            nc.gpsimd.dma_start(w2_t[:], d_w["w2"][:])

            # ---- constants ----
            mdec = smal.tile([P, 2, 4 * P], F16, name="mdec")
            nc.gpsimd.dma_start(mdec[:], d_mdec[:])
            hpair = smal.tile([P, P], F16, name="hpair")
            nc.sync.dma_start(hpair[:], d_hpair[:])
            dg = smal.tile([P, NPAIR, P], F16, name="dg")
            nc.sync.dma_start(dg[:], d_dg[:])
            identR = smal.tile([P, P], F16, name="identR")
            nc.gpsimd.dma_start(identR[:], d_identR[:])
            identF = smal.tile([P, P], F16, name="identF")
            nc.sync.dma_start(identF[:], d_identF[:])
            gdk = smal.tile([P, DIM], F16, name="gdk")
            nc.sync.dma_start(gdk[:], d_gdk[:])
            b1t = smal.tile([P, NDB], F32, name="b1t")
            nc.sync.dma_start(b1t[:], d_b1[:])
            b2t = smal.tile([P, NDB], F32, name="b2t")
            nc.sync.dma_start(b2t[:], d_b2[:])
            fcb = smal.tile([1, 1], F32, name="fcb")
            nc.sync.dma_start(fcb[:], d_fcb[:])
            jones = smal.tile([P, P], F16, name="jones")
            nc.gpsimd.memset(jones[:], 1.0)
            jones8 = smal.tile([P, 2, P], F8, name="jones8")
            nc.gpsimd.memset(jones8[:], 1.0)
            epsL = smal.tile([P, P], F16, name="epsL")
            nc.gpsimd.memset(epsL[:], 0.0025)
            epsR = smal.tile([P, 512], F16, name="epsR")
            nc.gpsimd.memset(epsR[:], 0.002)
            prime = smal.tile([1, 1], F16, name="prime")
            nc.vector.memset(prime[:], 0.0)
            nc.scalar.activation(prime[:], prime[:], AF.Silu)
            zacc = smal.tile([P, NCH], F32, name="zacc")

            qdec = acts.tile([P, NDB, SEQ], F16, tag="t_qdec", name="qdec")
            nc.gpsimd.dma_start(qdec[:], d_qdec[:])
            fct = acts.tile([P, NDB, SEQ], F16, tag="t_fct", name="fct")
            nc.gpsimd.dma_start(fct[:], d_fct[:])

            # ---- big activation tiles ----
            qt = acts.tile([P, NDB, SEQ], F16, tag="t_q", name="qt")
            qloc = acts.tile([P, NDB, SEQ], F16, tag="t_ql", name="qloc")
            kt = acts.tile([P, NDB, SEQ], F16, tag="t_k", name="kt")
            ktm = acts.tile([P, NTB, DIM], F16, tag="t_ktm", name="ktm")
            vtm = acts.tile([P, NTB, DIM], F16, tag="t_vtm", name="vtm")
            sw = acts.tile([P, NDB, SEQ], F16, tag="t_sw", name="sw")
            yt = acts.tile([P, NDB, SEQ], F16, tag="t_y", name="yt")
            ysq = acts.tile([P, NDB, SEQ], F16, tag="t_ysq", name="ysq")
            x1 = acts.tile([P, NDB, SEQ], F16, tag="t_x1", name="x1")
            h1 = acts.tile([P, NDB, SEQ], F8, tag="t_h1", name="h1")
            yt8 = acts.tile([P, NDB, SEQ], F8, tag="t_y8", name="yt8")
            x2 = acts.tile([P, NDB, SEQ], F16, tag="t_x2", name="x2")
            ssb = [acts.tile([P, 4 * P], F16, tag=f"t_ssb{i}", name=f"ssb{i}")
                   for i in (0, 1)]

            # -------- fp8 DoubleRow projection helpers (2 MMs per group) ----
            def proj_d(w_tile, src8, ch, writer, scope, resid=None,
                       resid_lhsT=None, tag="mm"):
                sl = slice(ch * 512, ch * 512 + 512)
                with nc.named_scope(scope):
                    for mb in range(NDB):
                        ps = pp.tile([P, 512], F32, tag=tag, bufs=2,
                                     name=f"{scope}_ps{mb}")
                        for c2 in range(2):
                            nc.tensor.matmul(
                                ps[:],
                                lhsT=w_tile[:, 2 * c2:2 * c2 + 2,
                                            mb * P:(mb + 1) * P],
                                rhs=src8[:, 2 * c2:2 * c2 + 2, sl],
                                start=(c2 == 0),
                                stop=(c2 == 1 and resid is None),
                                perf_mode=DR,
                            )
                        if resid is not None:
                            nc.tensor.matmul(ps[:], lhsT=resid_lhsT[:],
                                             rhs=resid[:, mb, sl],
                                             start=False, stop=True)
                        writer(mb, sl, ps)

            def proj_t(w_tile, src8, ch, writer, scope):
                with nc.named_scope(scope):
                    for tb in range(4 * ch, 4 * ch + 4):
                        ps = pp.tile([P, 512], F32, tag="mm", bufs=2,
                                     name=f"{scope}_ps{tb}")
                        for c2 in range(2):
                            nc.tensor.matmul(
                                ps[:],
                                lhsT=src8[:, 2 * c2:2 * c2 + 2,
                                          tb * P:(tb + 1) * P],
                                rhs=w_tile[:, 2 * c2:2 * c2 + 2, :],
                                start=(c2 == 0), stop=(c2 == 1),
                                perf_mode=DR,
                            )
                        writer(tb, ps)

            # ---------------- emit: gate + qkv projections ------------------
            def emit_projq(ch):
                def wr_q(mb, sl, ps):
                    nc.scalar.mul(qt[:, mb, sl], ps[:], 1.0 / WS)
                    nc.gpsimd.tensor_tensor(qloc[:, mb, sl], qt[:, mb, sl],
                                            qdec[:, mb, sl], OP.mult)
                proj_d(wq_t, xh18, ch, wr_q, f"projq{ch}")

            def emit_projk(ch):
                def wr_k(mb, sl, ps):
                    nc.vector.tensor_scalar_mul(kt[:, mb, sl], ps[:],
                                                1.0 / WS)
                proj_d(wk_t, xh18, ch, wr_k, f"projk{ch}")

            def emit_projv(ch):
                def wr_v(tb, ps):
                    nc.scalar.mul(vtm[:, tb, :], ps[:], 1.0 / WS)
                proj_t(wv_t, xh18, ch, wr_v, f"projv{ch}")

            def emit_projkt(ch):
                def wr_ktm(tb, ps):
                    # gdk carries the 1/WS descale (host-folded)
                    nc.vector.tensor_tensor(ktm[:, tb, :], ps[:], gdk[:],
                                            OP.mult)
                proj_t(wk_t, xh18, ch, wr_ktm, f"projkt{ch}")

            def emit_projg(ch):
                def wr_g(mb, sl, ps):
                    nc.scalar.activation(sw[:, mb, sl], ps[:], AF.Silu,
                                         scale=1.0 / WS)
                proj_d(wg_t, xh18, ch, wr_g, f"projg{ch}")

            # ---------------- attention block -------------------------------
            def attn_qb(qb):
                qsl = slice(qb * P, (qb + 1) * P)
                with nc.named_scope(f"attn{qb}"):
                    # diagonal-block scores, masked+decayed
                    stq = acts.tile([P, 2, 4 * P], F16, tag="t_st", bufs=4,
                                    name=f"stb{qb}")
                    sps = [pp.tile([P, 4 * P], F32, tag="sc", bufs=2,
                                   name=f"sps{qb}_{i}") for i in range(2)]
                    for t in range(NPAIR):
                        for i, r in ((0, 0), (1, 64)):
                            nc.tensor.matmul(
                                sps[i][:, t * P:(t + 1) * P],
                                lhsT=kt[r:r + 64, t, qsl],
                                rhs=qt[r:r + 64, t, qsl],
                                start=True, stop=True,
                            )
                    for i in range(2):
                        nc.vector.tensor_tensor(stq[:, i], sps[i][:],
                                                mdec[:, i], OP.mult)
                    # y psum for this block: pair-major [dv-pair, t, n]
                    yq = pp.tile([P, NPAIR, P], F32, tag="yq", bufs=2,
                                 name=f"yq{qb}")
                    sprev = ssb[(qb - 1) % 2]
                    for t in range(NPAIR):
                        if qb > 0:
                            for i in range(2):
                                pd = slice(i * 64, i * 64 + 64)
                                nc.tensor.matmul(
                                    yq[pd, t, :],
                                    lhsT=sprev[pd, t * P + i * 64:
                                               t * P + i * 64 + 64],
                                    rhs=qloc[pd, t, qsl],
                                    start=True, stop=False,
                                    skip_group_check=True,
                                )
                        for i in range(2):
                            pd = slice(i * 64, i * 64 + 64)
                            nc.tensor.matmul(
                                yq[pd, t, :],
                                lhsT=vtm[:, qb, t * P + i * 64:
                                         t * P + i * 64 + 64],
                                rhs=stq[:, i, t * P:(t + 1) * P],
                                start=(qb == 0), stop=True,
                                skip_group_check=True,
                            )
                    # evac y block into yt[:, pair, qb*128:+128]
                    nc.scalar.copy(yt[:, :, qsl], yq[:])
                    nc.scalar.activation(ysq[:, :, qsl], yq[:], AF.Square)
                    # state update on PE: S' = Dg * S + K^T V (per pair)
                    if qb < NTB - 1:
                        sp = pp.tile([P, 4 * P], F32, tag="st", bufs=2,
                                     name=f"sp{qb}")
                        for t in range(NPAIR):
                            tcol = slice(t * P, (t + 1) * P)
                            if qb > 0:
                                nc.tensor.matmul(
                                    sp[:, tcol], lhsT=dg[:, t, :],
                                    rhs=sprev[:, tcol],
                                    start=True, stop=False,
                                    skip_group_check=True,
                                )
                            nc.tensor.matmul(
                                sp[:, tcol],
                                lhsT=ktm[:, qb, tcol],
                                rhs=vtm[:, qb, tcol],
                                start=(qb == 0), stop=True,
                                skip_group_check=True,
                            )
                        nc.vector.tensor_scalar_mul(ssb[qb % 2][:], sp[:], 1.0)

            # ---------------- group norm + gate ------------------------------
            def gnorm_pair(t, ch):
                sl = slice(ch * 512, ch * 512 + 512)
                with nc.named_scope(f"gn{t}_{ch}"):
                    gps = pp.tile([P, 512], F32, tag="yq", bufs=2,
                                  name=f"gns{t}{ch}")
                    nc.tensor.matmul(gps[:], lhsT=hpair[:], rhs=yt[:, t, sl],
                                     start=True, stop=True)
                    gqs = pp.tile([P, 512], F32, tag="st", bufs=2,
                                  name=f"gnq{t}{ch}")
                    nc.tensor.matmul(gqs[:], lhsT=hpair[:], rhs=ysq[:, t, sl],
                                     start=True, stop=False)
                    nc.tensor.matmul(gqs[:], lhsT=epsL[:], rhs=epsR[:],
                                     start=False, stop=True)
                    gnm = acts.tile([P, 512], F16, tag="t_nmu", bufs=4,
                                    name=f"gnm{t}{ch}")
                    nc.scalar.mul(gnm[:], gps[:], -1.0 / HDIM)
                    mu2 = acts.tile([P, 512], F32, tag="t_mu2", bufs=3,
                                    name=f"gm2{t}{ch}")
                    nc.scalar.square(mu2[:], gnm[:])
                    gvar = acts.tile([P, 512], F32, tag="t_var", bufs=3,
                                     name=f"gvar{t}{ch}")
                    nc.vector.scalar_tensor_tensor(
                        gvar[:], gqs[:], 1.0 / HDIM, mu2[:],
                        OP.mult, OP.subtract)
                    grv = acts.tile([P, 512], F32, tag="t_rv", bufs=3,
                                    name=f"grv{t}{ch}")
                    nc.vector.reciprocal_approx_fast(grv[:], gvar[:])
                    grs = acts.tile([P, 512], F16, tag="t_rsd", bufs=4,
                                    name=f"grs{t}{ch}")
                    nc.scalar.sqrt(grs[:], grv[:])
                    nc.vector.tensor_tensor(yt[:, t, sl], yt[:, t, sl],
                                            gnm[:], OP.add)
                    nc.vector.tensor_tensor(yt[:, t, sl], yt[:, t, sl],
                                            grs[:], OP.mult)
                    nc.vector.tensor_tensor(yt8[:, t, sl], yt[:, t, sl],
                                            sw[:, t, sl], OP.mult)

            # ---------------- layer norm 2 (into h-path) ---------------------
            def ln2_ffn1(ch, tag="mm", ftag="mm"):
                sl = slice(ch * 512, ch * 512 + 512)
                with nc.named_scope(f"ln2c{ch}"):
                    psA = pp.tile([P, 512], F32, tag=tag, bufs=2,
                                  name=f"l2sA{ch}")
                    for kb in range(NDB):
                        nc.tensor.matmul(psA[:], lhsT=jones[:],
                                         rhs=x1[:, kb, sl],
                                         start=(kb == 0), stop=(kb == NDB - 1))
                    negmu = acts.tile([P, 512], F16, tag="t_nmu", bufs=4,
                                      name=f"l2nmu{ch}")
                    nc.vector.tensor_scalar_mul(negmu[:], psA[:], -1.0 / DIM)
                    xc = acts.tile([P, NDB, 512], F16, tag="t_xc", bufs=2,
                                   name=f"l2xc{ch}")
                    xcsq = acts.tile([P, NDB, 512], F8, tag="t_xcsq", bufs=2,
                                     name=f"l2xq{ch}")
                    for c in range(NDB):
                        nc.vector.tensor_tensor(xc[:, c], x1[:, c, sl],
                                                negmu[:], OP.add)
                        nc.scalar.activation(xcsq[:, c], xc[:, c], AF.Square,
                                             scale=0.5)
                    psB = pp.tile([P, 512], F32, tag=tag, bufs=2,
                                  name=f"l2sB{ch}")
                    for c2 in range(2):
                        nc.tensor.matmul(psB[:], lhsT=jones8[:],
                                         rhs=xcsq[:, 2 * c2:2 * c2 + 2, 0:512],
                                         start=(c2 == 0), stop=(c2 == 1),
                                         perf_mode=DR)
                    var = acts.tile([P, 512], F32, tag="t_var", bufs=3,
                                    name=f"l2var{ch}")
                    nc.vector.tensor_scalar(var[:], psB[:], 4.0 / DIM, EPS,
                                            OP.mult, OP.add)
                    rv = acts.tile([P, 512], F32, tag="t_rv", bufs=3,
                                   name=f"l2rv{ch}")
                    nc.vector.reciprocal_approx_fast(rv[:], var[:])
                    rstd = acts.tile([P, 512], F16, tag="t_rsd", bufs=4,
                                     name=f"l2rsd{ch}")
                    nc.scalar.sqrt(rstd[:], rv[:])
                    xh2 = acts.tile([P, NDB, 512], F8, tag="t_xh2", bufs=2,
                                    name=f"l2xh2{ch}")
                    for c in range(NDB):
                        nc.vector.tensor_tensor(xh2[:, c], xc[:, c], rstd[:],
                                                OP.mult)

                def wr_h1(mb, sl2, ps):
                    # h1 = HS * relu(W1 xh2 + b1); b1t is host-scaled by HS
                    nc.scalar.activation(h1[:, mb, sl2], ps[:], AF.Relu,
                                         bias=b1t[:, mb:mb + 1],
                                         scale=HS / WS)

                with nc.named_scope(f"ffn1{ch}"):
                    for mb in range(NDB):
                        ps = pp.tile([P, 512], F32, tag=ftag, bufs=2,
                                     name=f"ffn1{ch}_ps{mb}")
                        for c2 in range(2):
                            nc.tensor.matmul(
                                ps[:],
                                lhsT=w1_t[:, 2 * c2:2 * c2 + 2,
                                          mb * P:(mb + 1) * P],
                                rhs=xh2[:, 2 * c2:2 * c2 + 2, 0:512],
                                start=(c2 == 0), stop=(c2 == 1),
                                perf_mode=DR,
                            )
                        wr_h1(mb, sl, ps)

            # ---------------- projo / ffn2 with residual ---------------------
            def emit_projo(ch, tag="mm"):
                def wr_o(mb, sl, ps):
                    nc.scalar.mul(x1[:, mb, sl], ps[:], 1.0 / WS)
                proj_d(wo_t, yt8, ch, wr_o, f"projo{ch}", resid=xh1,
                       resid_lhsT=identR, tag=tag)

            def emit_ffn2(ch, tag="mm"):
                def wr_x2(mb, sl, ps):
                    nc.scalar.activation(x2[:, mb, sl], ps[:], AF.Identity,
                                         bias=b2t[:, mb:mb + 1],
                                         scale=1.0 / (WS * HS))
                proj_d(w2_t, h1, ch, wr_x2, f"ffn2{ch}", resid=x1,
                       resid_lhsT=identF, tag=tag)

            # ---------------- LN3 folded into classifier ---------------------
            def ln3_cls(ch, tag="mm"):
                sl = slice(ch * 512, ch * 512 + 512)
                with nc.named_scope(f"ln3c{ch}"):
                    psA = pp.tile([P, 512], F32, tag=tag, bufs=2,
                                  name=f"l3sA{ch}")
                    for kb in range(NDB):
                        nc.tensor.matmul(psA[:], lhsT=jones[:],
                                         rhs=x2[:, kb, sl],
                                         start=(kb == 0), stop=(kb == NDB - 1))
                    negmu = acts.tile([P, 512], F16, tag="t_nmu", bufs=4,
                                      name=f"l3nmu{ch}")
                    nc.vector.tensor_scalar_mul(negmu[:], psA[:], -1.0 / DIM)
                    xc = acts.tile([P, NDB, 512], F16, tag="t_xc", bufs=2,
                                   name=f"l3xc{ch}")
                    xcsq = acts.tile([P, NDB, 512], F8, tag="t_xcsq", bufs=2,
                                     name=f"l3xq{ch}")
                    pr = acts.tile([P, NDB, 512], F16, tag="t_pr", bufs=2,
                                   name=f"l3pr{ch}")
                    for c in range(NDB):
                        nc.vector.tensor_tensor(xc[:, c], x2[:, c, sl],
                                                negmu[:], OP.add)
                        nc.scalar.activation(xcsq[:, c], xc[:, c], AF.Square,
                                             scale=0.5)
                        nc.vector.tensor_tensor(pr[:, c], xc[:, c],
                                                fct[:, c, sl], OP.mult)
                    psB = pp.tile([P, 512], F32, tag=tag, bufs=2,
                                  name=f"l3sB{ch}")
                    for c2 in range(2):
                        nc.tensor.matmul(psB[:], lhsT=jones8[:],
                                         rhs=xcsq[:, 2 * c2:2 * c2 + 2, 0:512],
                                         start=(c2 == 0), stop=(c2 == 1),
                                         perf_mode=DR)
                    psU = pp.tile([P, 512], F32, tag=tag, bufs=2,
                                  name=f"l3sU{ch}")
                    for kb in range(NDB):
                        nc.tensor.matmul(psU[:], lhsT=jones[:],
                                         rhs=pr[:, kb],
                                         start=(kb == 0), stop=(kb == NDB - 1))
                    var = acts.tile([P, 512], F32, tag="t_var", bufs=3,
                                    name=f"l3var{ch}")
                    nc.vector.tensor_scalar(var[:], psB[:], 4.0 / DIM, EPS,
                                            OP.mult, OP.add)
                    rv = acts.tile([P, 512], F32, tag="t_rv", bufs=3,
                                   name=f"l3rv{ch}")
                    nc.vector.reciprocal_approx_fast(rv[:], var[:])
                    rstd = acts.tile([P, 512], F16, tag="t_rsd", bufs=4,
                                     name=f"l3rsd{ch}")
                    nc.scalar.sqrt(rstd[:], rv[:])
                    zj = acts.tile([P, 512], F16, tag="t_zj", bufs=2,
                                   name=f"l3zj{ch}")
                    nc.vector.scalar_tensor_tensor(
                        zj[:], psU[:], 1.0, rstd[:], OP.mult, OP.mult,
                        accum_out=zacc[:, ch:ch + 1])

            # ================= program order =================================
            emit_projq(0)
            emit_projk(0)
            emit_projv(0)
            emit_projkt(0)
            emit_projg(0)
            emit_projg(1)

            attn_qb(0)
            emit_projq(1)
            attn_qb(1)
            emit_projk(1)
            attn_qb(2)
            emit_projv(1)
            attn_qb(3)
            emit_projkt(1)
            attn_qb(4)
            gnorm_pair(0, 0)
            attn_qb(5)
            gnorm_pair(1, 0)
            attn_qb(6)
            gnorm_pair(2, 0)
            attn_qb(7)
            gnorm_pair(3, 0)

            emit_projo(0, tag="mm")
            gnorm_pair(0, 1)
            gnorm_pair(1, 1)
            gnorm_pair(2, 1)
            gnorm_pair(3, 1)
            emit_projo(1, tag="sc")
            ln2_ffn1(0, tag="mm", ftag="sc")
            ln2_ffn1(1, tag="yq", ftag="st")
            emit_ffn2(0, tag="mm")
            ln3_cls(0, tag="sc")
            emit_ffn2(1, tag="yq")
            ln3_cls(1, tag="st")

            # ================= final classifier reduce =======================
            with nc.named_scope("classfin"):
                zred = smal.tile([P, 1], F32, name="zred")
                nc.vector.tensor_reduce(zred[:], zacc[:],
                                        axis=mybir.AxisListType.X, op=OP.add)
                logit = smal.tile([1, 1], F32, name="logit")
                nc.scalar.activation(logit[:], zred[0:1, :], AF.Identity,
                                     bias=fcb[:])
                nc.sync.dma_start(d_out[:], logit[:])

    nc.finalize()
    return nc


# ---------------- host-side input prep ----------------

def prep_in_maps(inputs):
    ids = np.asarray(inputs["ids"])[NVOCAB - 1]                      # [B, S]
    tab = np.asarray(inputs["emb_tables"], np.float32)[NVOCAB - 1]   # [V, D]
    pos = np.asarray(inputs["pos_emb"], np.float32)[NVOCAB - 1]      # [S, D]
    gamv = np.asarray(inputs["ln_gamma"], np.float32)
    betv = np.asarray(inputs["ln_beta"], np.float32)

    f8np = mybir.dt.np(F8)

    def wblocks(w):  # [Din, Dout] -> [P, NDB, Dout], fp8 scaled by WS
        wb = np.ascontiguousarray(w.reshape(NDB, P, DIM).transpose(1, 0, 2))
        return np.clip(wb * WS, -448.0, 448.0).astype(f8np)

    def pvec(v):  # [D] -> [P, NDB]
        return np.ascontiguousarray(v.reshape(NDB, P).T, np.float32)

    gammas = 1.0 - 2.0 ** (-5.0 - np.arange(HEADS, dtype=np.float64))
    nloc = np.arange(P, dtype=np.float64)
    # head of (partition p, block c) is 2c + p//64
    hmap = 2 * np.arange(NDB)[None, :] + (np.arange(P) // HDIM)[:, None]
    lng = np.log(gammas)[hmap]                                   # [P, NDB]
    # qdec[p, c, n] = g^{(n%128)+1}
    nmod = (np.arange(SEQ) % P + 1).astype(np.float64)
    qdec = np.exp(lng[:, :, None] * nmod[None, None, :]).astype(np.float16)
    # gdecK[m, d] = g_{d//64}^{127-m} / 8, with the fp8 1/WS descale folded
    gd = np.exp(np.log(gammas)[None, np.arange(DIM) // HDIM]
                * (127.0 - nloc)[:, None]) / 8.0 / WS
    gdecK = gd.astype(np.float16)
    # mdec[m, i, t*128+n] = g_{2t+i}^{n-m}/8 if n>=m else 0
    mdec = np.zeros((P, 2, NPAIR * P), np.float64)
    for t in range(NPAIR):
        for i in range(2):
            g = gammas[2 * t + i]
            m = nloc[:, None]
            n = nloc[None, :]
            mdec[:, i, t * P:(t + 1) * P] = np.where(
                n >= m, g ** (n - m), 0.0) / 8.0
    mdec = mdec.astype(np.float16)
    hp = np.zeros((P, P), np.float16)
    hp[0:64, 0:64] = 1.0
    hp[64:128, 64:128] = 1.0
    # dg[p, t, q] = (p==q) * gammas[2t + p//64]^128
    dgm = np.zeros((P, NPAIR, P), np.float64)
    for t in range(NPAIR):
        gvals = gammas[2 * t + (np.arange(P) // 64)] ** 128.0
        dgm[np.arange(P), t, np.arange(P)] = gvals
    dgm = dgm.astype(np.float16)
    id_r = (np.eye(P) * WS).astype(np.float16)
    id_f = (np.eye(P) * WS * HS).astype(np.float16)

    # classifier: fold gamma into fc weights, beta into the bias constant
    fcw = np.asarray(inputs["fc_W"], np.float32).reshape(SEQ, DIM)
    fcg = fcw * gamv[None, :]                      # [S, D]
    fct = np.ascontiguousarray(
        fcg.reshape(SEQ, NDB, P).transpose(2, 1, 0)
    ).astype(np.float16)
    fcb_c = float(np.asarray(inputs["fc_b"], np.float32).reshape(())
                  + (fcw * betv[None, :]).sum())

    # LN2 affine folded into W1/b1: h1 = HS*relu(W1g^T xh2_plain + b1g)
    w1 = np.asarray(inputs["W1"], np.float32)
    w1g = gamv[:, None] * w1
    b1g = (np.asarray(inputs["b1"], np.float32) + betv @ w1) * HS

    common = {
        "wq": wblocks(np.asarray(inputs["Wq"], np.float32)),
        "wk": wblocks(np.asarray(inputs["Wk"], np.float32)),
        "wv": wblocks(np.asarray(inputs["Wv"], np.float32)),
        "wg": wblocks(np.asarray(inputs["Wg"], np.float32)),
        "wo": wblocks(np.asarray(inputs["Wo"], np.float32)),
        "w1": wblocks(w1g),
        "w2": wblocks(np.asarray(inputs["W2"], np.float32)),
        "qdec": qdec,
        "gdk": gdecK,
        "mdec": mdec,
        "hpair": hp,
        "dg": dgm,
        "identR": id_r,
        "identF": id_f,
        "b1t": pvec(b1g),
        "b2t": pvec(np.asarray(inputs["b2"], np.float32)),
        "fcT": fct,
        "fcb": np.asarray(fcb_c, np.float32).reshape(1, 1),
    }

    in_maps = []
    for b in range(BATCH):
        emb = tab[ids[b]] + pos                                  # [S, D]
        mu = emb.mean(-1, keepdims=True)
        var = emb.var(-1, keepdims=True)
        x = (emb - mu) / np.sqrt(var + EPS) * gamv[None, :] + betv[None, :]
        m = dict(common)
        xt = np.ascontiguousarray(x.T.reshape(NDB, P, SEQ).transpose(1, 0, 2))
        m["xh1"] = xt.astype(np.float16)
        m["xh18"] = np.clip(xt, -448.0, 448.0).astype(f8np)
        in_maps.append(m)
    return in_maps


_NC_CACHE = {}


def get_nc():
    if "nc" not in _NC_CACHE:
        _NC_CACHE["nc"] = build_nc()
    return _NC_CACHE["nc"]


def _sigmoid(x):
    return 1.0 / (1.0 + np.exp(-x))


def kernel(**inputs) -> np.ndarray:
    nc = get_nc()
    in_maps = prep_in_maps(inputs)
    res = run_bass_kernel_spmd(nc, in_maps, core_ids=list(range(NCORES)))
    logits = np.stack(
        [np.asarray(res.results[b]["out"]).reshape(()) for b in range(BATCH)]
    ).reshape(BATCH, 1)
    return _sigmoid(logits.astype(np.float64)).astype(np.float32)


# revision 50
# speedup vs baseline: 1.0031x; 1.0031x over previous
"""Trainium2 Bass kernel for nn_ACPClassifier (RetNet-style block + classifier).

v7 design (~131us vs 200-238us v2 baseline). Key points:
- Only the last vocab iteration matters (x overwritten each pass); data
  parallel over batch, one batch element per NeuronCore.
- Host: embedding gather + pos add + LN1 (same DMA bytes as shipping emb);
  final sigmoid on host (kernel returns the logit).
- All seven projections run as fp8e4 DoubleRow matmuls (weights x256,
  descale folded into the PSUM-evac activation scale): 2 MMs per 512-deep
  contraction instead of 4. Attention stays fp16.
- Act-table discipline: Silu gates + reciprocal_approx_fast(DVE)+Sqrt for
  every rstd -> 3 table loads total (v2 had 29 = 45us); silu set primed at
  t=0 via a 1-element dummy op on the vector+scalar queues.
- Residual adds folded into projection matmul groups via scaled-identity
  lhsT (identR=256*I, identF=4096*I match the fp8 scale chain).
- Retention state recurrence on the PE: per-pair merged KV matmul plus a
  diagonal-decay matmul (gamma^128 per head), one state-evac copy per block.
  Score/cross MMs ordered so 64-row halves land in disjoint PE row/col
  groups and run concurrently.
- Group norm with uncentered stats: y and y^2 both evacuated from PSUM
  during the attention phase, so both hpair stat matmuls fire at tail start.
- LN2/LN3 sumsq via fp8 jones DoubleRow on Square(xc/2) with 4/D var
  rescale; LN3 folded into the classifier dot on centered x (no wfc term).
- DMAs spread across sync/scalar/gpsimd queues; chunk-1 projections
  interleaved with chunk-0 attention to keep the PE HAM-warm.
"""

import numpy as np

import concourse.bacc as bacc
import concourse.mybir as mybir
from concourse.bass_utils import run_bass_kernel_spmd
from concourse.tile import TileContext

F32 = mybir.dt.float32
F16 = mybir.dt.float16
F8 = mybir.dt.float8e4
DR = mybir.MatmulPerfMode.DoubleRow
WS = 256.0   # fp8 weight scale
HS = 16.0    # h1 activation scale
AF = mybir.ActivationFunctionType
OP = mybir.AluOpType

DIM, SEQ, HEADS, HDIM, BATCH, VOCAB, NVOCAB = 512, 1024, 8, 64, 8, 1024, 3
EPS = 1e-5
P = 128
NDB = DIM // P    # 4 d-blocks (also head pairs)
NTB = SEQ // P    # 8 token blocks
NCH = SEQ // 512  # 2 free-dim chunks of 512
NPAIR = 4
NCORES = 8


def build_nc():
    nc = bacc.Bacc(
        "TRN2",
        target_bir_lowering=False,
        debug=False,
        enable_asserts=False,
        num_devices=NCORES,
    )

    # ---- DRAM parameters (per-core inputs) ----
    d_xh1 = nc.declare_dram_parameter("xh1", [P, NDB, SEQ], F16, isOutput=False)
    d_w = {
        k: nc.declare_dram_parameter(k, [P, NDB, DIM], F8, isOutput=False)
        for k in ("wq", "wk", "wv", "wg", "wo", "w1", "w2")
    }
    d_xh18 = nc.declare_dram_parameter("xh18", [P, NDB, SEQ], F8, isOutput=False)
    d_qdec = nc.declare_dram_parameter("qdec", [P, NDB, SEQ], F16, isOutput=False)
    d_gdk = nc.declare_dram_parameter("gdk", [P, DIM], F16, isOutput=False)
    d_mdec = nc.declare_dram_parameter("mdec", [P, 2, 4 * P], F16, isOutput=False)
    d_hpair = nc.declare_dram_parameter("hpair", [P, P], F16, isOutput=False)
    d_dg = nc.declare_dram_parameter("dg", [P, NPAIR, P], F16, isOutput=False)
    d_identR = nc.declare_dram_parameter("identR", [P, P], F16, isOutput=False)
    d_identF = nc.declare_dram_parameter("identF", [P, P], F16, isOutput=False)
    d_fct = nc.declare_dram_parameter("fcT", [P, NDB, SEQ], F16, isOutput=False)
    d_b1 = nc.declare_dram_parameter("b1t", [P, NDB], F32, isOutput=False)
    d_b2 = nc.declare_dram_parameter("b2t", [P, NDB], F32, isOutput=False)
    d_fcb = nc.declare_dram_parameter("fcb", [1, 1], F32, isOutput=False)
    d_out = nc.declare_dram_parameter("out", [1, 1], F32, isOutput=True)

    with TileContext(nc) as tc:
        from contextlib import ExitStack

        ctx = ExitStack()
        with ctx:
            acts = ctx.enter_context(tc.tile_pool(name="acts", bufs=1))
            wts = ctx.enter_context(tc.tile_pool(name="wts", bufs=1))
            smal = ctx.enter_context(tc.tile_pool(name="smal", bufs=1))
            pp = ctx.enter_context(tc.tile_pool(name="pp", bufs=1, space="PSUM"))

            # ---- weights on the projq critical path first ----
            def load_w(key):
                t = wts.tile([P, NDB, DIM], F8, tag=f"t_w_{key}",
                             name=f"w_{key}")
                nc.sync.dma_start(t[:], d_w[key][:])
                return t

            wq_t = load_w("wq")
            xh18 = acts.tile([P, NDB, SEQ], F8, tag="t_xh18", name="xh18")
            nc.scalar.dma_start(xh18[:], d_xh18[:])
            xh1 = acts.tile([P, NDB, SEQ], F16, tag="t_xh1", name="xh1")
            nc.gpsimd.dma_start(xh1[:], d_xh1[:])
            wk_t = wts.tile([P, NDB, DIM], F8, tag="t_w_wk", name="w_wk")
            nc.gpsimd.dma_start(wk_t[:], d_w["wk"][:])
            wv_t = wts.tile([P, NDB, DIM], F8, tag="t_w_wv", name="w_wv")
            nc.scalar.dma_start(wv_t[:], d_w["wv"][:])
            wg_t = wts.tile([P, NDB, DIM], F8, tag="t_w_wg", name="w_wg")
            nc.sync.dma_start(wg_t[:], d_w["wg"][:])
            wo_t = wts.tile([P, NDB, DIM], F8, tag="t_w_wo", name="w_wo")
            nc.scalar.dma_start(wo_t[:], d_w["wo"][:])
            w1_t = wts.tile([P, NDB, DIM], F8, tag="t_w_w1", name="w_w1")
            nc.sync.dma_start(w1_t[:], d_w["w1"][:])
            w2_t = wts.tile([P, NDB, DIM], F8, tag="t_w_w2", name="w_w2")
            nc.gpsimd.dma_start(w2_t[:], d_w["w2"][:])

            # ---- constants ----
            mdec = smal.tile([P, 2, 4 * P], F16, name="mdec")
            nc.gpsimd.dma_start(mdec[:], d_mdec[:])
            hpair = smal.tile([P, P], F16, name="hpair")
            nc.sync.dma_start(hpair[:], d_hpair[:])
            dg = smal.tile([P, NPAIR, P], F16, name="dg")
            nc.sync.dma_start(dg[:], d_dg[:])
            identR = smal.tile([P, P], F16, name="identR")
            nc.gpsimd.dma_start(identR[:], d_identR[:])
            identF = smal.tile([P, P], F16, name="identF")
            nc.sync.dma_start(identF[:], d_identF[:])
            gdk = smal.tile([P, DIM], F16, name="gdk")
            nc.sync.dma_start(gdk[:], d_gdk[:])
            b1t = smal.tile([P, NDB], F32, name="b1t")
            nc.sync.dma_start(b1t[:], d_b1[:])
            b2t = smal.tile([P, NDB], F32, name="b2t")
            nc.sync.dma_start(b2t[:], d_b2[:])
            fcb = smal.tile([1, 1], F32, name="fcb")
            nc.sync.dma_start(fcb[:], d_fcb[:])
            jones = smal.tile([P, P], F16, name="jones")
            nc.gpsimd.memset(jones[:], 1.0)
            jones8 = smal.tile([P, 2, P], F8, name="jones8")
            nc.gpsimd.memset(jones8[:], 1.0)
            epsL = smal.tile([P, P], F16, name="epsL")
            nc.gpsimd.memset(epsL[:], 0.0025)
            epsR = smal.tile([P, 512], F16, name="epsR")
            nc.gpsimd.memset(epsR[:], 0.002)
            prime = smal.tile([1, 1], F16, name="prime")
            nc.vector.memset(prime[:], 0.0)
            nc.scalar.activation(prime[:], prime[:], AF.Silu)
            zacc = smal.tile([P, NCH], F32, name="zacc")

            qdec = acts.tile([P, NDB, SEQ], F16, tag="t_qdec", name="qdec")
            nc.gpsimd.dma_start(qdec[:], d_qdec[:])
            fct = acts.tile([P, NDB, SEQ], F16, tag="t_fct", name="fct")
            nc.gpsimd.dma_start(fct[:], d_fct[:])

            # ---- big activation tiles ----
            qt = acts.tile([P, NDB, SEQ], F16, tag="t_q", name="qt")
            qloc = acts.tile([P, NDB, SEQ], F16, tag="t_ql", name="qloc")
            kt = acts.tile([P, NDB, SEQ], F16, tag="t_k", name="kt")
            ktm = acts.tile([P, NTB, DIM], F16, tag="t_ktm", name="ktm")
            vtm = acts.tile([P, NTB, DIM], F16, tag="t_vtm", name="vtm")
            sw = acts.tile([P, NDB, SEQ], F16, tag="t_sw", name="sw")
            yt = acts.tile([P, NDB, SEQ], F16, tag="t_y", name="yt")
            ysq = acts.tile([P, NDB, SEQ], F16, tag="t_ysq", name="ysq")
            x1 = acts.tile([P, NDB, SEQ], F16, tag="t_x1", name="x1")
            h1 = acts.tile([P, NDB, SEQ], F8, tag="t_h1", name="h1")
            yt8 = acts.tile([P, NDB, SEQ], F8, tag="t_y8", name="yt8")
            x2 = acts.tile([P, NDB, SEQ], F16, tag="t_x2", name="x2")
            ssb = [acts.tile([P, 4 * P], F16, tag=f"t_ssb{i}", name=f"ssb{i}")
                   for i in (0, 1)]

            # -------- fp8 DoubleRow projection helpers (2 MMs per group) ----
            def proj_d(w_tile, src8, ch, writer, scope, resid=None,
                       resid_lhsT=None, tag="mm"):
                sl = slice(ch * 512, ch * 512 + 512)
                with nc.named_scope(scope):
                    for mb in range(NDB):
                        ps = pp.tile([P, 512], F32, tag=tag, bufs=2,
                                     name=f"{scope}_ps{mb}")
                        for c2 in range(2):
                            nc.tensor.matmul(
                                ps[:],
                                lhsT=w_tile[:, 2 * c2:2 * c2 + 2,
                                            mb * P:(mb + 1) * P],
                                rhs=src8[:, 2 * c2:2 * c2 + 2, sl],
                                start=(c2 == 0),
                                stop=(c2 == 1 and resid is None),
                                perf_mode=DR,
                            )
                        if resid is not None:
                            nc.tensor.matmul(ps[:], lhsT=resid_lhsT[:],
                                             rhs=resid[:, mb, sl],
                                             start=False, stop=True)
                        writer(mb, sl, ps)

            def proj_t(w_tile, src8, ch, writer, scope):
                with nc.named_scope(scope):
                    for tb in range(4 * ch, 4 * ch + 4):
                        ps = pp.tile([P, 512], F32, tag="mm", bufs=2,
                                     name=f"{scope}_ps{tb}")
                        for c2 in range(2):
                            nc.tensor.matmul(
                                ps[:],
                                lhsT=src8[:, 2 * c2:2 * c2 + 2,
                                          tb * P:(tb + 1) * P],
                                rhs=w_tile[:, 2 * c2:2 * c2 + 2, :],
                                start=(c2 == 0), stop=(c2 == 1),
                                perf_mode=DR,
                            )
                        writer(tb, ps)

            # ---------------- emit: gate + qkv projections ------------------
            def emit_projq(ch):
                def wr_q(mb, sl, ps):
                    nc.scalar.mul(qt[:, mb, sl], ps[:], 1.0 / WS)
                    nc.gpsimd.tensor_tensor(qloc[:, mb, sl], qt[:, mb, sl],
                                            qdec[:, mb, sl], OP.mult)
                proj_d(wq_t, xh18, ch, wr_q, f"projq{ch}")

            def emit_projk(ch):
                def wr_k(mb, sl, ps):
                    nc.vector.tensor_scalar_mul(kt[:, mb, sl], ps[:],
                                                1.0 / WS)
                proj_d(wk_t, xh18, ch, wr_k, f"projk{ch}")

            def emit_projv(ch):
                def wr_v(tb, ps):
                    nc.scalar.mul(vtm[:, tb, :], ps[:], 1.0 / WS)
                proj_t(wv_t, xh18, ch, wr_v, f"projv{ch}")

            def emit_projkt(ch):
                def wr_ktm(tb, ps):
                    # gdk carries the 1/WS descale (host-folded)
                    nc.vector.tensor_tensor(ktm[:, tb, :], ps[:], gdk[:],
                                            OP.mult)
                proj_t(wk_t, xh18, ch, wr_ktm, f"projkt{ch}")

            def emit_projg(ch):
                def wr_g(mb, sl, ps):
                    nc.scalar.activation(sw[:, mb, sl], ps[:], AF.Silu,
                                         scale=1.0 / WS)
                proj_d(wg_t, xh18, ch, wr_g, f"projg{ch}")

            # ---------------- attention block -------------------------------
            def attn_qb(qb):
                qsl = slice(qb * P, (qb + 1) * P)
                with nc.named_scope(f"attn{qb}"):
                    # diagonal-block scores, masked+decayed
                    stq = acts.tile([P, 2, 4 * P], F16, tag="t_st", bufs=2,
                                    name=f"stb{qb}")
                    sps = [pp.tile([P, 4 * P], F32, tag="sc", bufs=2,
                                   name=f"sps{qb}_{i}") for i in range(2)]
                    for t in range(NPAIR):
                        for i, r in ((0, 0), (1, 64)):
                            nc.tensor.matmul(
                                sps[i][:, t * P:(t + 1) * P],
                                lhsT=kt[r:r + 64, t, qsl],
                                rhs=qt[r:r + 64, t, qsl],
                                start=True, stop=True,
                            )
                    for i in range(2):
                        nc.vector.tensor_tensor(stq[:, i], sps[i][:],
                                                mdec[:, i], OP.mult)
                    # y psum for this block: pair-major [dv-pair, t, n]
                    yq = pp.tile([P, NPAIR, P], F32, tag="yq", bufs=2,
                                 name=f"yq{qb}")
                    sprev = ssb[(qb - 1) % 2]
                    for t in range(NPAIR):
                        if qb > 0:
                            for i in range(2):
                                pd = slice(i * 64, i * 64 + 64)
                                nc.tensor.matmul(
                                    yq[pd, t, :],
                                    lhsT=sprev[pd, t * P + i * 64:
                                               t * P + i * 64 + 64],
                                    rhs=qloc[pd, t, qsl],
                                    start=True, stop=False,
                                    skip_group_check=True,
                                )
                        for i in range(2):
                            pd = slice(i * 64, i * 64 + 64)
                            nc.tensor.matmul(
                                yq[pd, t, :],
                                lhsT=vtm[:, qb, t * P + i * 64:
                                         t * P + i * 64 + 64],
                                rhs=stq[:, i, t * P:(t + 1) * P],
                                start=(qb == 0), stop=True,
                                skip_group_check=True,
                            )
                    # evac y block into yt[:, pair, qb*128:+128]
                    nc.scalar.copy(yt[:, :, qsl], yq[:])
                    nc.scalar.activation(ysq[:, :, qsl], yq[:], AF.Square)
                    # state update on PE: S' = Dg * S + K^T V (per pair)
                    if qb < NTB - 1:
                        sp = pp.tile([P, 4 * P], F32, tag="st", bufs=2,
                                     name=f"sp{qb}")
                        for t in range(NPAIR):
                            tcol = slice(t * P, (t + 1) * P)
                            if qb > 0:
                                nc.tensor.matmul(
                                    sp[:, tcol], lhsT=dg[:, t, :],
                                    rhs=sprev[:, tcol],
                                    start=True, stop=False,
                                    skip_group_check=True,
                                )
                            nc.tensor.matmul(
                                sp[:, tcol],
                                lhsT=ktm[:, qb, tcol],
                                rhs=vtm[:, qb, tcol],
                                start=(qb == 0), stop=True,
                                skip_group_check=True,
                            )
                        nc.vector.tensor_scalar_mul(ssb[qb % 2][:], sp[:], 1.0)

            # ---------------- group norm + gate ------------------------------
            def gnorm_pair(t, ch):
                sl = slice(ch * 512, ch * 512 + 512)
                with nc.named_scope(f"gn{t}_{ch}"):
                    gps = pp.tile([P, 512], F32, tag="yq", bufs=2,
                                  name=f"gns{t}{ch}")
                    nc.tensor.matmul(gps[:], lhsT=hpair[:], rhs=yt[:, t, sl],
                                     start=True, stop=True)
                    gqs = pp.tile([P, 512], F32, tag="st", bufs=2,
                                  name=f"gnq{t}{ch}")
                    nc.tensor.matmul(gqs[:], lhsT=hpair[:], rhs=ysq[:, t, sl],
                                     start=True, stop=False)
                    nc.tensor.matmul(gqs[:], lhsT=epsL[:], rhs=epsR[:],
                                     start=False, stop=True)
                    gnm = acts.tile([P, 512], F16, tag="t_nmu", bufs=4,
                                    name=f"gnm{t}{ch}")
                    nc.scalar.mul(gnm[:], gps[:], -1.0 / HDIM)
                    mu2 = acts.tile([P, 512], F32, tag="t_mu2", bufs=2,
                                    name=f"gm2{t}{ch}")
                    nc.scalar.square(mu2[:], gnm[:])
                    gvar = acts.tile([P, 512], F32, tag="t_var", bufs=2,
                                     name=f"gvar{t}{ch}")
                    nc.vector.scalar_tensor_tensor(
                        gvar[:], gqs[:], 1.0 / HDIM, mu2[:],
                        OP.mult, OP.subtract)
                    grv = acts.tile([P, 512], F32, tag="t_rv", bufs=2,
                                    name=f"grv{t}{ch}")
                    nc.vector.reciprocal_approx_fast(grv[:], gvar[:])
                    grs = acts.tile([P, 512], F16, tag="t_rsd", bufs=4,
                                    name=f"grs{t}{ch}")
                    nc.scalar.sqrt(grs[:], grv[:])
                    nc.vector.tensor_tensor(yt[:, t, sl], yt[:, t, sl],
                                            gnm[:], OP.add)
                    nc.vector.tensor_tensor(yt[:, t, sl], yt[:, t, sl],
                                            grs[:], OP.mult)
                    nc.vector.tensor_tensor(yt8[:, t, sl], yt[:, t, sl],
                                            sw[:, t, sl], OP.mult)

            # ---------------- layer norm 2 (into h-path) ---------------------
            def ln2_ffn1(ch, tag="mm", ftag="mm"):
                sl = slice(ch * 512, ch * 512 + 512)
                with nc.named_scope(f"ln2c{ch}"):
                    psA = pp.tile([P, 512], F32, tag=tag, bufs=2,
                                  name=f"l2sA{ch}")
                    for kb in range(NDB):
                        nc.tensor.matmul(psA[:], lhsT=jones[:],
                                         rhs=x1[:, kb, sl],
                                         start=(kb == 0), stop=(kb == NDB - 1))
                    negmu = acts.tile([P, 512], F16, tag="t_nmu", bufs=4,
                                      name=f"l2nmu{ch}")
                    nc.vector.tensor_scalar_mul(negmu[:], psA[:], -1.0 / DIM)
                    xc = acts.tile([P, NDB, 512], F16, tag="t_xc", bufs=2,
                                   name=f"l2xc{ch}")
                    xcsq = acts.tile([P, NDB, 512], F8, tag="t_xcsq", bufs=2,
                                     name=f"l2xq{ch}")
                    for c in range(NDB):
                        nc.vector.tensor_tensor(xc[:, c], x1[:, c, sl],
                                                negmu[:], OP.add)
                        nc.scalar.activation(xcsq[:, c], xc[:, c], AF.Square,
                                             scale=0.5)
                    psB = pp.tile([P, 512], F32, tag=tag, bufs=2,
                                  name=f"l2sB{ch}")
                    for c2 in range(2):
                        nc.tensor.matmul(psB[:], lhsT=jones8[:],
                                         rhs=xcsq[:, 2 * c2:2 * c2 + 2, 0:512],
                                         start=(c2 == 0), stop=(c2 == 1),
                                         perf_mode=DR)
                    var = acts.tile([P, 512], F32, tag="t_var", bufs=2,
                                    name=f"l2var{ch}")
                    nc.vector.tensor_scalar(var[:], psB[:], 4.0 / DIM, EPS,
                                            OP.mult, OP.add)
                    rv = acts.tile([P, 512], F32, tag="t_rv", bufs=2,
                                   name=f"l2rv{ch}")
                    nc.vector.reciprocal_approx_fast(rv[:], var[:])
                    rstd = acts.tile([P, 512], F16, tag="t_rsd", bufs=4,
                                     name=f"l2rsd{ch}")
                    nc.scalar.sqrt(rstd[:], rv[:])
                    xh2 = acts.tile([P, NDB, 512], F8, tag="t_xh2", bufs=2,
                                    name=f"l2xh2{ch}")
                    for c in range(NDB):
                        nc.vector.tensor_tensor(xh2[:, c], xc[:, c], rstd[:],
                                                OP.mult)

                def wr_h1(mb, sl2, ps):
                    # h1 = HS * relu(W1 xh2 + b1); b1t is host-scaled by HS
                    nc.scalar.activation(h1[:, mb, sl2], ps[:], AF.Relu,
                                         bias=b1t[:, mb:mb + 1],
                                         scale=HS / WS)

                with nc.named_scope(f"ffn1{ch}"):
                    for mb in range(NDB):
                        ps = pp.tile([P, 512], F32, tag=ftag, bufs=2,
                                     name=f"ffn1{ch}_ps{mb}")
                        for c2 in range(2):
                            nc.tensor.matmul(
                                ps[:],
                                lhsT=w1_t[:, 2 * c2:2 * c2 + 2,
                                          mb * P:(mb + 1) * P],
                                rhs=xh2[:, 2 * c2:2 * c2 + 2, 0:512],
                                start=(c2 == 0), stop=(c2 == 1),
                                perf_mode=DR,
                            )
                        wr_h1(mb, sl, ps)

            # ---------------- projo / ffn2 with residual ---------------------
            def emit_projo(ch, tag="mm"):
                def wr_o(mb, sl, ps):
                    nc.scalar.mul(x1[:, mb, sl], ps[:], 1.0 / WS)
                proj_d(wo_t, yt8, ch, wr_o, f"projo{ch}", resid=xh1,
                       resid_lhsT=identR, tag=tag)

            def emit_ffn2(ch, tag="mm"):
                def wr_x2(mb, sl, ps):
                    nc.scalar.activation(x2[:, mb, sl], ps[:], AF.Identity,
                                         bias=b2t[:, mb:mb + 1],
                                         scale=1.0 / (WS * HS))
                proj_d(w2_t, h1, ch, wr_x2, f"ffn2{ch}", resid=x1,
                       resid_lhsT=identF, tag=tag)

            # ---------------- LN3 folded into classifier ---------------------
            def ln3_cls(ch, tag="mm"):
                sl = slice(ch * 512, ch * 512 + 512)
                with nc.named_scope(f"ln3c{ch}"):
                    psA = pp.tile([P, 512], F32, tag=tag, bufs=2,
                                  name=f"l3sA{ch}")
                    for kb in range(NDB):
                        nc.tensor.matmul(psA[:], lhsT=jones[:],
                                         rhs=x2[:, kb, sl],
                                         start=(kb == 0), stop=(kb == NDB - 1))
                    negmu = acts.tile([P, 512], F16, tag="t_nmu", bufs=4,
                                      name=f"l3nmu{ch}")
                    nc.vector.tensor_scalar_mul(negmu[:], psA[:], -1.0 / DIM)
                    xc = acts.tile([P, NDB, 512], F16, tag="t_xc", bufs=2,
                                   name=f"l3xc{ch}")
                    xcsq = acts.tile([P, NDB, 512], F8, tag="t_xcsq", bufs=2,
                                     name=f"l3xq{ch}")
                    pr = acts.tile([P, NDB, 512], F16, tag="t_pr", bufs=2,
                                   name=f"l3pr{ch}")
                    for c in range(NDB):
                        nc.vector.tensor_tensor(xc[:, c], x2[:, c, sl],
                                                negmu[:], OP.add)
                        nc.scalar.activation(xcsq[:, c], xc[:, c], AF.Square,
                                             scale=0.5)
                        nc.vector.tensor_tensor(pr[:, c], xc[:, c],
                                                fct[:, c, sl], OP.mult)
                    psB = pp.tile([P, 512], F32, tag=tag, bufs=2,
                                  name=f"l3sB{ch}")
                    for c2 in range(2):
                        nc.tensor.matmul(psB[:], lhsT=jones8[:],
                                         rhs=xcsq[:, 2 * c2:2 * c2 + 2, 0:512],
                                         start=(c2 == 0), stop=(c2 == 1),
                                         perf_mode=DR)
                    psU = pp.tile([P, 512], F32, tag=tag, bufs=2,
                                  name=f"l3sU{ch}")
                    for kb in range(NDB):
                        nc.tensor.matmul(psU[:], lhsT=jones[:],
                                         rhs=pr[:, kb],
                                         start=(kb == 0), stop=(kb == NDB - 1))
                    var = acts.tile([P, 512], F32, tag="t_var", bufs=2,
                                    name=f"l3var{ch}")
                    nc.vector.tensor_scalar(var[:], psB[:], 4.0 / DIM, EPS,
                                            OP.mult, OP.add)
                    rv = acts.tile([P, 512], F32, tag="t_rv", bufs=2,
                                   name=f"l3rv{ch}")
                    nc.vector.reciprocal_approx_fast(rv[:], var[:])
                    rstd = acts.tile([P, 512], F16, tag="t_rsd", bufs=4,
                                     name=f"l3rsd{ch}")
                    nc.scalar.sqrt(rstd[:], rv[:])
                    zj = acts.tile([P, 512], F16, tag="t_zj", bufs=2,
                                   name=f"l3zj{ch}")
                    nc.vector.scalar_tensor_tensor(
                        zj[:], psU[:], 1.0, rstd[:], OP.mult, OP.mult,
                        accum_out=zacc[:, ch:ch + 1])

            # ================= program order =================================
            emit_projq(0)
            emit_projk(0)
            emit_projv(0)
            emit_projkt(0)
            emit_projg(0)
            emit_projg(1)

            attn_qb(0)
            emit_projq(1)
            attn_qb(1)
            emit_projk(1)
            attn_qb(2)
            emit_projv(1)
            attn_qb(3)
            emit_projkt(1)
            attn_qb(4)
            gnorm_pair(0, 0)
            attn_qb(5)
            gnorm_pair(1, 0)
            attn_qb(6)
            gnorm_pair(2, 0)
            attn_qb(7)
            gnorm_pair(3, 0)

            emit_projo(0, tag="mm")
            gnorm_pair(0, 1)
            gnorm_pair(1, 1)
            gnorm_pair(2, 1)
            gnorm_pair(3, 1)
            emit_projo(1, tag="sc")
            ln2_ffn1(0, tag="mm", ftag="sc")
            ln2_ffn1(1, tag="yq", ftag="st")
            emit_ffn2(0, tag="mm")
            ln3_cls(0, tag="sc")
            emit_ffn2(1, tag="yq")
            ln3_cls(1, tag="st")

            # ================= final classifier reduce =======================
            with nc.named_scope("classfin"):
                zred = smal.tile([P, 1], F32, name="zred")
                nc.vector.tensor_reduce(zred[:], zacc[:],
                                        axis=mybir.AxisListType.X, op=OP.add)
                logit = smal.tile([1, 1], F32, name="logit")
                nc.scalar.activation(logit[:], zred[0:1, :], AF.Identity,
                                     bias=fcb[:])
                nc.sync.dma_start(d_out[:], logit[:])

    nc.finalize()
    return nc


# ---------------- host-side input prep ----------------

def prep_in_maps(inputs):
    ids = np.asarray(inputs["ids"])[NVOCAB - 1]                      # [B, S]
    tab = np.asarray(inputs["emb_tables"], np.float32)[NVOCAB - 1]   # [V, D]
    pos = np.asarray(inputs["pos_emb"], np.float32)[NVOCAB - 1]      # [S, D]
    gamv = np.asarray(inputs["ln_gamma"], np.float32)
    betv = np.asarray(inputs["ln_beta"], np.float32)

    f8np = mybir.dt.np(F8)

    def wblocks(w):  # [Din, Dout] -> [P, NDB, Dout], fp8 scaled by WS
        wb = np.ascontiguousarray(w.reshape(NDB, P, DIM).transpose(1, 0, 2))
        return np.clip(wb * WS, -448.0, 448.0).astype(f8np)

    def pvec(v):  # [D] -> [P, NDB]
        return np.ascontiguousarray(v.reshape(NDB, P).T, np.float32)

    gammas = 1.0 - 2.0 ** (-5.0 - np.arange(HEADS, dtype=np.float64))
    nloc = np.arange(P, dtype=np.float64)
    # head of (partition p, block c) is 2c + p//64
    hmap = 2 * np.arange(NDB)[None, :] + (np.arange(P) // HDIM)[:, None]
    lng = np.log(gammas)[hmap]                                   # [P, NDB]
    # qdec[p, c, n] = g^{(n%128)+1}
    nmod = (np.arange(SEQ) % P + 1).astype(np.float64)
    qdec = np.exp(lng[:, :, None] * nmod[None, None, :]).astype(np.float16)
    # gdecK[m, d] = g_{d//64}^{127-m} / 8, with the fp8 1/WS descale folded
    gd = np.exp(np.log(gammas)[None, np.arange(DIM) // HDIM]
                * (127.0 - nloc)[:, None]) / 8.0 / WS
    gdecK = gd.astype(np.float16)
    # mdec[m, i, t*128+n] = g_{2t+i}^{n-m}/8 if n>=m else 0
    mdec = np.zeros((P, 2, NPAIR * P), np.float64)
    for t in range(NPAIR):
        for i in range(2):
            g = gammas[2 * t + i]
            m = nloc[:, None]
            n = nloc[None, :]
            mdec[:, i, t * P:(t + 1) * P] = np.where(
                n >= m, g ** (n - m), 0.0) / 8.0
    mdec = mdec.astype(np.float16)
    hp = np.zeros((P, P), np.float16)
    hp[0:64, 0:64] = 1.0
    hp[64:128, 64:128] = 1.0
    # dg[p, t, q] = (p==q) * gammas[2t + p//64]^128
    dgm = np.zeros((P, NPAIR, P), np.float64)
    for t in range(NPAIR):
        gvals = gammas[2 * t + (np.arange(P) // 64)] ** 128.0
        dgm[np.arange(P), t, np.arange(P)] = gvals
    dgm = dgm.astype(np.float16)
    id_r = (np.eye(P) * WS).astype(np.float16)
    id_f = (np.eye(P) * WS * HS).astype(np.float16)

    # classifier: fold gamma into fc weights, beta into the bias constant
    fcw = np.asarray(inputs["fc_W"], np.float32).reshape(SEQ, DIM)
    fcg = fcw * gamv[None, :]                      # [S, D]
    fct = np.ascontiguousarray(
        fcg.reshape(SEQ, NDB, P).transpose(2, 1, 0)
    ).astype(np.float16)
    fcb_c = float(np.asarray(inputs["fc_b"], np.float32).reshape(())
                  + (fcw * betv[None, :]).sum())

    # LN2 affine folded into W1/b1: h1 = HS*relu(W1g^T xh2_plain + b1g)
    w1 = np.asarray(inputs["W1"], np.float32)
    w1g = gamv[:, None] * w1
    b1g = (np.asarray(inputs["b1"], np.float32) + betv @ w1) * HS

    common = {
        "wq": wblocks(np.asarray(inputs["Wq"], np.float32)),
        "wk": wblocks(np.asarray(inputs["Wk"], np.float32)),
        "wv": wblocks(np.asarray(inputs["Wv"], np.float32)),
        "wg": wblocks(np.asarray(inputs["Wg"], np.float32)),
        "wo": wblocks(np.asarray(inputs["Wo"], np.float32)),
        "w1": wblocks(w1g),
        "w2": wblocks(np.asarray(inputs["W2"], np.float32)),
        "qdec": qdec,
        "gdk": gdecK,
        "mdec": mdec,
        "hpair": hp,
        "dg": dgm,
        "identR": id_r,
        "identF": id_f,
        "b1t": pvec(b1g),
        "b2t": pvec(np.asarray(inputs["b2"], np.float32)),
        "fcT": fct,
        "fcb": np.asarray(fcb_c, np.float32).reshape(1, 1),
    }

    in_maps = []
    for b in range(BATCH):
        emb = tab[ids[b]] + pos                                  # [S, D]
        mu = emb.mean(-1, keepdims=True)
        var = emb.var(-1, keepdims=True)
        x = (emb - mu) / np.sqrt(var + EPS) * gamv[None, :] + betv[None, :]
        m = dict(common)
        xt = np.ascontiguousarray(x.T.reshape(NDB, P, SEQ).transpose(1, 0, 2))
        m["xh1"] = xt.astype(np.float16)
        m["xh18"] = np.clip(xt, -448.0, 448.0).astype(f8np)
        in_maps.append(m)
    return in_maps


_NC_CACHE = {}


def get_nc():
    if "nc" not in _NC_CACHE:
        _NC_CACHE["nc"] = build_nc()
    return _NC_CACHE["nc"]


def _sigmoid(x):
    return 1.0 / (1.0 + np.exp(-x))


def kernel(**inputs) -> np.ndarray:
    nc = get_nc()
    in_maps = prep_in_maps(inputs)
    res = run_bass_kernel_spmd(nc, in_maps, core_ids=list(range(NCORES)))
    logits = np.stack(
        [np.asarray(res.results[b]["out"]).reshape(()) for b in range(BATCH)]
    ).reshape(BATCH, 1)
    return _sigmoid(logits.astype(np.float64)).astype(np.float32)
